# revision 15
# baseline (speedup 1.0000x reference)
"""Bayesian linear layer (Monte-Carlo reparameterized GEMM) on 8 Trainium2 cores.

y[s,b,o] = sum_i x[b,i] * (w_mu[o,i] + exp(w_lsigma[o,i]) * r1[s,o,i]) + b_mu[o]
           + exp(b_lsigma[o]) * r2[s,o]

Sharding: samples s split across the 8 cores (8 samples/core); x and the
(mu, lsigma) parameters replicated.

Per-core device kernel:
  - stream r1[s] tiles (SWDGE queue), PE-transpose them, fuse
    w_sT = E^T o r1^T + w_mu^T on DVE (constants resident in [i,o] layout)
  - GEMM y[s] = x @ w_s^T as float32r (FP22) matmuls: lhsT = x^T tiles
    (streamed, shared across a sample pair), rhs = w_sT, k-accumulated in PSUM
  - evict PSUM via ACT copy + DVE adds (bias fused), DMA out on the
    Scalar HWDGE queue

When w_lsigma is a constant fill (E = exp(w_lsigma) scalar c — true for the
reference inputs), the host folds c into x and w_mu:
    y = (c*x) @ (r1^T + (w_mu/c)^T) + bias
so the per-sample transform is a single DVE add per tile.
"""

import sys

if "/opt/trn_rl_repo" not in sys.path:
    sys.path.insert(0, "/opt/trn_rl_repo")

from contextlib import ExitStack

import numpy as np

import concourse.bass as bass  # noqa: F401
import concourse.tile as tile
from concourse import bacc, mybir
from concourse.bass_utils import run_bass_kernel_spmd
from concourse.masks import make_identity

P = 128
N_IN = 1024
N_OUT = 1024
BATCH = 4096
S = 64
NCORES = 8
SC = S // NCORES  # samples per core
KT = N_IN // P  # 8 k-tiles
BT = BATCH // P  # 32 b-tiles
OW = 512  # o chunk (one PSUM bank of fp32)
OH = N_OUT // OW  # 2 o-halves

F32 = mybir.dt.float32
F32R = mybir.dt.float32r

_CACHE = {}


def build_bass(scalar_e: bool):
    nc = bacc.Bacc("TRN2", target_bir_lowering=False, debug=False)

    xT = nc.dram_tensor("xT", [N_IN, BATCH], F32, kind="ExternalInput").ap()
    wmuT = nc.dram_tensor("wmuT", [N_IN, N_OUT], F32, kind="ExternalInput").ap()
    r1s = nc.dram_tensor("r1s", [SC, N_OUT, N_IN], F32, kind="ExternalInput").ap()
    biass = nc.dram_tensor("biass", [SC, N_OUT], F32, kind="ExternalInput").ap()
    if not scalar_e:
        ET = nc.dram_tensor("ET", [N_IN, N_OUT], F32, kind="ExternalInput").ap()
    y = nc.dram_tensor("y", [SC, BATCH, N_OUT], F32, kind="ExternalOutput").ap()

    with tile.TileContext(nc) as tc, ExitStack() as ctx:
        const = ctx.enter_context(tc.tile_pool(name="const", bufs=1))
        xt_pool = ctx.enter_context(tc.tile_pool(name="xt", bufs=3))
        wst_pool = ctx.enter_context(tc.tile_pool(name="wst", bufs=2))
        r1_pool = ctx.enter_context(tc.tile_pool(name="r1", bufs=4 if scalar_e else 3))
        y_pool = ctx.enter_context(tc.tile_pool(name="yp", bufs=3))
        bias_pool = ctx.enter_context(tc.tile_pool(name="bias", bufs=2))
        pt_pool = ctx.enter_context(tc.tile_pool(name="pt", bufs=2, space="PSUM"))
        pm_pool = ctx.enter_context(tc.tile_pool(name="pm", bufs=6, space="PSUM"))

        ident_f32 = const.tile([P, P], F32)
        make_identity(nc, ident_f32[:])
        ident = const.tile([P, P], F32R)
        nc.vector.tensor_copy(ident[:], ident_f32[:])

        # constants resident in [i, o] layout: [p, k, o] with i = k*P + p
        wmuT_sb = const.tile([P, KT, N_OUT], F32)
        for k in range(KT):
            nc.sync.dma_start(wmuT_sb[:, k, :], wmuT[k * P : (k + 1) * P, :])
        if not scalar_e:
            ET_sb = const.tile([P, KT, N_OUT], F32)
            for k in range(KT):
                nc.sync.dma_start(ET_sb[:, k, :], ET[k * P : (k + 1) * P, :])

        def make_transform(s):
            """Transform for sample s as a list of closures: emit them
            interleaved into the previous sample's matmul sweep so slab DMAs
            spread out and the transposes hide inside the PE stream."""
            wst = wst_pool.tile([P, KT, N_OUT], F32R, tag="wst", name=f"wst_{s}")
            state = {"bias": None}
            slabs = {}

            def mk_bias():
                def f():
                    bm = bias_pool.tile([P, N_OUT], F32, tag="bias")
                    nc.sync.dma_start(
                        bm[:], biass[s][None, :].broadcast_to((P, N_OUT))
                    )
                    state["bias"] = bm

                return f

            def mk_slab(oh, h):
                def f():
                    slab = r1_pool.tile(
                        [P, 2, N_IN], F32R, tag="r1", name=f"r1_{s}_{oh}_{h}"
                    )
                    base = oh * OW + h * 2 * P
                    nc.gpsimd.dma_start(
                        slab[:],
                        r1s[s, base : base + 2 * P, :]
                        .rearrange("(t p) i -> p t i", p=P)
                        .bitcast(F32R),
                    )
                    slabs[(oh, h)] = slab

                return f

            def mk_unit(oh, it):
                def f():
                    osl = slice(oh * OW, (oh + 1) * OW)
                    ps = pt_pool.tile([P, OW], F32R, tag="pt")
                    for ot in range(4):
                        nc.tensor.transpose(
                            ps[:, ot * P : (ot + 1) * P],
                            slabs[(oh, ot // 2)][:, ot % 2, it * P : (it + 1) * P],
                            ident[:],
                        )
                    if scalar_e:
                        # wst = r1^T + (w_mu/c)^T   (c folded into x on host)
                        nc.vector.tensor_add(wst[:, it, osl], ps[:], wmuT_sb[:, it, osl])
                    else:
                        nc.vector.tensor_mul(wst[:, it, osl], ps[:], ET_sb[:, it, osl])
                        nc.vector.tensor_add(
                            wst[:, it, osl], wst[:, it, osl], wmuT_sb[:, it, osl]
                        )

                return f

            closures = [mk_bias()]
            for oh in range(OH):
                closures.append(mk_slab(oh, 0))
                closures.append(mk_slab(oh, 1))
                for it in range(KT):
                    closures.append(mk_unit(oh, it))
            return wst, state, closures

        def emit_sweep(s, wst, bias_state, next_closures):
            ci = 0
            for bt in range(BT):
                xt = xt_pool.tile([P, KT, P], F32R, tag="xt")
                xslab = xT[:, bt * P : (bt + 1) * P].rearrange("(k p) b -> p k b", p=P)
                nc.sync.dma_start(xt[:], xslab.bitcast(F32R))
                pms = {}
                for oh in range(OH):
                    pms[oh] = pm_pool.tile([P, OW], F32, tag="pm", name=f"pm_{oh}")
                # k-major so the stationary x tile is shared by both o-halves
                for k in range(KT):
                    lhsT = xt[:, k, :]
                    for oh in range(OH):
                        nc.tensor.matmul(
                            pms[oh][:],
                            lhsT,
                            wst[:, k, oh * OW : (oh + 1) * OW],
                            start=(k == 0),
                            stop=(k == KT - 1),
                        )
                bm = bias_state["bias"]
                yt = y_pool.tile([P, N_OUT], F32, tag="y")
                # o-half 0: ACT copy + DVE bias add; o-half 1: DVE fused add
                nc.scalar.copy(yt[:, 0:OW], pms[0][:])
                nc.vector.tensor_add(yt[:, 0:OW], yt[:, 0:OW], bm[:, 0:OW])
                nc.vector.tensor_add(yt[:, OW:], pms[1][:], bm[:, OW:])
                nc.scalar.dma_start(y[s, bt * P : (bt + 1) * P, :], yt[:])
                # interleave next sample's transform into this sweep
                if bt >= 1 and ci < len(next_closures):
                    next_closures[ci]()
                    ci += 1
            for f in next_closures[ci:]:
                f()

        wst, bias_state, closures = make_transform(0)
        for f in closures:
            f()
        for s in range(SC):
            if s + 1 < SC:
                wst_next, bias_next, closures_next = make_transform(s + 1)
            else:
                wst_next, bias_next, closures_next = None, None, []
            emit_sweep(s, wst, bias_state, closures_next)
            wst, bias_state = wst_next, bias_next

    nc.compile()
    return nc


def _get_nc(scalar_e: bool):
    key = ("nc", scalar_e)
    if key not in _CACHE:
        _CACHE[key] = build_bass(scalar_e)
    return _CACHE[key]


def _prep(x, w_mu, w_lsigma, b_mu, b_lsigma, r1, r2):
    """Host-side marshalling. Returns (scalar_e, per-core-constant input dict)."""
    bias = (b_mu[None, :] + np.exp(b_lsigma)[None, :] * r2).astype(np.float32)
    scalar_e = bool(np.all(w_lsigma == w_lsigma.flat[0]))
    if scalar_e:
        c = np.float32(np.exp(w_lsigma.flat[0]))
        xT = np.ascontiguousarray((c * x).T.astype(np.float32))
        wmuT = np.ascontiguousarray((w_mu / c).T.astype(np.float32))
        consts = {"xT": xT, "wmuT": wmuT}
    else:
        xT = np.ascontiguousarray(x.T)
        wmuT = np.ascontiguousarray(w_mu.T)
        ET = np.ascontiguousarray(np.exp(w_lsigma).T.astype(np.float32))
        consts = {"xT": xT, "wmuT": wmuT, "ET": ET}
    return scalar_e, consts, bias


def kernel(x, w_mu, w_lsigma, b_mu, b_lsigma, r1, r2, N_samples):
    x = np.asarray(x, dtype=np.float32)
    w_mu = np.asarray(w_mu, dtype=np.float32)
    w_lsigma = np.asarray(w_lsigma, dtype=np.float32)
    b_mu = np.asarray(b_mu, dtype=np.float32)
    b_lsigma = np.asarray(b_lsigma, dtype=np.float32)
    r1 = np.asarray(r1, dtype=np.float32)
    r2 = np.asarray(r2, dtype=np.float32)
    assert x.shape == (BATCH, N_IN) and r1.shape == (S, N_OUT, N_IN)

    scalar_e, consts, bias = _prep(x, w_mu, w_lsigma, b_mu, b_lsigma, r1, r2)
    nc = _get_nc(scalar_e)

    in_maps = []
    for c in range(NCORES):
        sl = slice(c * SC, (c + 1) * SC)
        in_maps.append(
            dict(
                consts,
                r1s=np.ascontiguousarray(r1[sl]),
                biass=np.ascontiguousarray(bias[sl]),
            )
        )

    res = run_bass_kernel_spmd(nc, in_maps, core_ids=list(range(NCORES)))
    out = np.concatenate([res.results[c]["y"] for c in range(NCORES)], axis=0)
    return out
